# revision 42
# baseline (speedup 1.0000x reference)
"""Trainium2 Bass kernel for the branched cross-attention processor.

Problem (full shapes):
  hidden_states [4, 4096, 1280], encoder_hidden_states [4, 77, 2048],
  id_embedding [2, 32, 2048], Wq/Wout [1280,1280], Wk/Wv/Wid_k/Wid_v
  [2048,1280], bout [1280].  20 heads, dh=64.  Output [4, 4096, 1280].

Sharding: data-parallel over (batch, seq-half): core c handles batch c//2,
query rows (c%2)*2048 : (c%2+1)*2048.  K/V (109 keys) are computed
per-core for its batch.  All queries are independent (full cross
attention), so no collectives are needed beyond the 2-core KV exchange.

Single fused pipeline (fp16 matmul operands, fp32 PSUM accumulation):

Q projection runs c-chunk-major (4 chunks of 512 query columns) with the
j-groups inside, so the first useful matmul only needs wq[0] plus the
first 512-column slice of hsT (~1.6 MB) instead of all of hsT (5.6 MB).
hsT arrives as per-(i, c0) slices followed by per-i remainders; the wq
stream rides one j ahead of consumption.  The KV projection is
PAIR-SPLIT (even core computes the encoder projection, odd core the id
projection; same SPMD code, different weight data) and exchanged with a
2-core DRAM AllGather; the 5 kv sub-chunks interleave between j-groups
paced by their weight DMAs, finishing at the end of chunk 2 so the
exchange + kT transposes hide under chunk 3's matmuls.

Attention + output projection then stream through the same PE queue.
Per head-pair: 2 row-group scores matmuls (K=64, rows 0:64 / 64:128)
into one 2-bank psum tile so they carry a single WAR wait; exp with
gap-mask bias on ACT; 2 col-group PV matmuls + 2 col-group
ones-denominator matmuls (alternating col strips 0:64 / 64:128);
reciprocal + normalize on DVE.  attnT is written back into the qT tiles
(read-then-overwrite per chunk; saves 5MB SBUF).  As soon as a chunk's
10 head-pairs finish, its 4 query tiles of the output projection are
emitted interleaved with the next chunk's attention so the
exp/normalize work hides under out-proj matmuls and the PE never idles.

Key layout: [0:77]=encoder keys, [77:96]=zero gap (exp bias -1e30),
[96:128]=id keys.
"""

import os
import sys
import types

import numpy as np

# ---------------------------------------------------------------------------
# problem constants (hardcoded; kernel.py must be self-contained)
# ---------------------------------------------------------------------------
B = 4
S = 4096
H = 1280
C = 2048
TE = 77          # encoder tokens
TI = 32          # id tokens
HEADS = 20
DH = 64          # head dim
P = 128
L = 109          # TE + TI
LP = 128         # padded key count: [0:77]=ehs, [77:96]=gap, [96:128]=id
GAP0, GAP1 = TE, P - TI   # 77, 96
SC = 2048        # seq rows per core
NJ = H // P      # 10
NI = C // P      # 16
NCH = SC // 512  # 4 sq-chunks of 512
NT = SC // P     # 16 sq-tiles of 128
SCALE = 1.0 / 8.0
NCORES = 8
MCHUNKS = [(0, 512), (512, 512), (1024, 256)]

_NC_CACHE = {}


def _ensure_axon_hooks():
    """The image's antenv lacks axon_hooks; synthesize it so NTFF profiling
    (trace=True) works when test.py asks for it.  Harmless if unused."""
    if "antenv.axon_hooks" in sys.modules:
        return
    try:
        import antenv
        from trn_agent_boot.trn_boot import _ntff_profile_via_ctypes

        hook = _ntff_profile_via_ctypes("/opt/axon/libaxon_pjrt.so")
        m = types.ModuleType("antenv.axon_hooks")
        m.get_axon_ntff_profile_hook = lambda: hook
        m.set_axon_ntff_profile_hook = lambda h: None
        sys.modules["antenv.axon_hooks"] = m
        antenv.axon_hooks = m
    except Exception:
        pass


def build_nc():
    """Build + compile the per-core Bass program (SPMD: same NEFF, 8 cores)."""
    if "nc" in _NC_CACHE:
        return _NC_CACHE["nc"]

    import concourse.bass as bass
    import concourse.tile as tile
    from concourse import bacc, mybir
    from concourse.bass import ts

    F32 = mybir.dt.float32
    R = mybir.dt.float16      # matmul operand dtype (1 cyc/row, 10-bit mantissa)
    EXP = mybir.ActivationFunctionType.Exp

    nc = bacc.Bacc("TRN2", target_bir_lowering=False, debug=False, num_devices=NCORES)

    hsT = nc.dram_tensor("hsT", [H, SC], R, kind="ExternalInput").ap()
    xkvTp = nc.dram_tensor("xkvTp", [P, NI * LP], R, kind="ExternalInput").ap()
    wqp = nc.dram_tensor("wqp", [NJ, P, H], R, kind="ExternalInput").ap()
    # per-core kv weights: even cores get [Wk|Wv], odd cores [Wid_k|Wid_v];
    # each core computes only its projection and the pair exchanges results.
    # 10 sub-chunks of 256 output columns so the weight stream arrives (and
    # the exchange can trigger) as early as possible.
    wkvp = nc.dram_tensor("wkvp", [10, P, NI * 256], R, kind="ExternalInput").ap()
    woutT = nc.dram_tensor("woutT", [H, H], R, kind="ExternalInput").ap()
    boutb = nc.dram_tensor("boutb", [P, H], F32, kind="ExternalInput").ap()
    # fp16 output: the values come out of fp32 psum and round once on the
    # final store (~5e-4 rel), halving the 10.5MB/core output write; the
    # host gather upcasts back to fp32.
    out = nc.dram_tensor("out", [SC, H], R, kind="ExternalOutput").ap()
    kvstore = nc.dram_tensor("kvstore", [P, 5 * 512], R, kind="Internal").ap()
    kvgather = nc.dram_tensor("kvgather", [2, P, 5 * 512], R, kind="Internal").ap()
    # tiny tensors for a warm-up AllGather that absorbs the CC stream's
    # one-time setup latency long before the real exchange
    ccwarm_in = nc.dram_tensor("ccwarm_in", [P, 16], R, kind="Internal").ap()
    ccwarm_out = nc.dram_tensor("ccwarm_out", [2, P, 16], R, kind="Internal").ap()

    with tile.TileContext(nc) as tc:
        with tc.tile_pool(name="pers", bufs=1) as pers:
            # ---- persistent constants / arrays --------------------------------
            ones_mat = pers.tile([P, P], R, tag="ones_mat")
            nc.vector.memset(ones_mat[:, :], 1.0)
            bias_col = pers.tile([P, 1], F32, tag="bias_col")
            # engine ops need 32-aligned start partitions: write the gap
            # as [64:96] then restore [64:77]; later writes overwrite cleanly.
            nc.vector.memset(bias_col[:, :], 0.0)
            nc.vector.memset(bias_col[64:GAP1, :], -1e30)
            nc.vector.memset(bias_col[64:GAP0, :], 0.0)
            kT_sb = [pers.tile([P, LP], R, tag=f"kT{j}", name=f"kT{j}") for j in range(NJ)]
            # merged pair-exchanged [k~|v~] (cols 0:1280 k, 1280:2560 v)
            kvall = pers.tile([P, 5 * 512], R, tag="kvall")
            v_sb = kvall[:, 1280:2560]
            # fp16 identity for PE-transpose of the kT tiles (the PE builds
            # kT right when it would otherwise stall on the exchange, ~128
            # cycles per tile vs ~1.2us per DMA-transpose issue)
            ident16 = pers.tile([P, P], R, tag="ident16")
            from concourse import masks as _masks
            _masks.make_identity(nc, ident16[:, :])
            # qT doubles as attnT in phase 2 (normalize overwrites each
            # chunk after its scores matmul has consumed it).
            qT_sb = [pers.tile([P, SC], R, tag=f"qT{j}", name=f"qT{j}") for j in range(NJ)]
            wout_sb = [pers.tile([P, H], R, tag=f"wout{i}", name=f"wout{i}") for i in range(NJ)]
            boutb_sb = pers.tile([P, H], F32, tag="boutb")

            # ---- phase 1: q projection chunk-major + kv proj, DMA-paced -------
            with (
                tc.tile_pool(name="phq", bufs=1) as phq,
                tc.tile_pool(name="wkvs", bufs=6) as wkvs,
                tc.tile_pool(name="psq", bufs=4, space="PSUM") as psq,
                tc.tile_pool(name="pskv", bufs=2, space="PSUM") as pskv,
            ):
                hsT_sb = [pers.tile([P, SC], R, tag=f"hsT{i}", name=f"hsT{i}") for i in range(NJ)]
                wq_sb = [pers.tile([P, H], R, tag=f"wq{j}", name=f"wq{j}") for j in range(NJ)]
                xkv_all = phq.tile([P, NI * LP], R, tag="xkv_all")
                kvmine = phq.tile([P, 5 * 512], R, tag="kvmine")
                warm_mov = phq.tile([P, 512], R, tag="warm_mov")
                nc.vector.memset(warm_mov[:, :], 0.0)

                # PE warmup: ~8 matmuls on constants lift the PE p-state /
                # HAM clock gate while the first input DMAs land.  The psum
                # tile is never read; psq reuse orders it before q group 0.
                wps = psq.tile([P, 512], F32, tag="qps", name="warm_ps")
                for w in range(8):
                    nc.tensor.matmul(wps[:, :], ones_mat[:, :], warm_mov[:, :],
                                     start=(w == 0), stop=(w == 7))

                # DMA issue order is (approximate) arrival order.  Critical
                # path first: wq0/wq1 + the c0 slices of hsT so the first
                # j-groups start ~9us in; then the remaining wq stream, the
                # hsT remainders, and the kv weights behind them.
                nc.sync.dma_start(out=wq_sb[0][:, :], in_=wqp[0])
                nc.sync.dma_start(out=wq_sb[1][:, :], in_=wqp[1])
                for i in range(NJ):
                    nc.sync.dma_start(out=hsT_sb[i][:, 0:512], in_=hsT[ts(i, P), 0:512])
                for j in range(2, NJ):
                    nc.sync.dma_start(out=wq_sb[j][:, :], in_=wqp[j])
                for i in range(NJ):
                    nc.sync.dma_start(out=hsT_sb[i][:, 512:1024], in_=hsT[ts(i, P), 512:1024])
                nc.sync.dma_start(out=xkv_all[:, :], in_=xkvTp)

                kv_tiles = {}

                def issue_kv(n):
                    t = wkvs.tile([P, NI * 256], R, tag="wkv", name=f"wkv{n}")
                    nc.sync.dma_start(out=t[:, :], in_=wkvp[n])
                    kv_tiles[n] = t

                issue_kv(0)
                issue_kv(1)
                issue_kv(2)
                issue_kv(3)
                issue_kv(4)
                issue_kv(5)
                for i in range(NJ):
                    nc.sync.dma_start(out=hsT_sb[i][:, 1024:SC], in_=hsT[ts(i, P), 1024:SC])

                def kv_sub(n):
                    kvw = kv_tiles[n]
                    ps = pskv.tile([P, 256], F32, tag="kvps", name="kvps")
                    for i in range(NI):
                        nc.tensor.matmul(
                            ps[:, :], xkv_all[:, ts(i, LP)], kvw[:, ts(i, 256)],
                            start=(i == 0), stop=(i == NI - 1),
                        )
                    # evacuate on DVE (idle in phase 1) so the store isn't
                    # queued behind ACT's qT copies
                    nc.vector.tensor_copy(kvmine[:, ts(n, 256)], ps[:, :])
                    # stream my projection to HBM incrementally so the
                    # exchange can start right after the last sub-chunk
                    nc.sync.dma_start(out=kvstore[:, ts(n, 256)],
                                      in_=kvmine[:, ts(n, 256)])
                    if n + 6 < 10:
                        issue_kv(n + 6)

                # per-(c, j) hooks: all 10 kv sub-chunks run inside chunk 1
                # (their 256-col weight mega-DMAs arrive ~27-57us), so the
                # exchange triggers as early as the PE pace allows (~85us)
                # and the collective (high run-to-run latency variance)
                # completes before the q stream runs dry at ~122us.
                def hooks(c, j):
                    if (c, j) == (0, 1):
                        # warm-up AllGather on 4KB: absorbs the CC stream's
                        # ~11us one-time trigger/setup latency early, off the
                        # critical path
                        nc.gpsimd.collective_compute(
                            "AllGather", mybir.AluOpType.bypass,
                            replica_groups=[[0, 1], [2, 3], [4, 5], [6, 7]],
                            ins=[ccwarm_in], outs=[ccwarm_out],
                        )
                    elif c == 1:
                        kv_sub(j)
                        if j == 9:
                            # 2-core AllGather: slot 0 = even core ([Wk|Wv]
                            # proj, valid rows 0:96), slot 1 = odd ([Wid_*],
                            # rows 96:128)
                            nc.gpsimd.collective_compute(
                                "AllGather", mybir.AluOpType.bypass,
                                replica_groups=[[0, 1], [2, 3], [4, 5], [6, 7]],
                                ins=[kvstore], outs=[kvgather],
                            )
                    elif (c, j) == (2, 1):
                        # split across DGE queue sets so the two reads run
                        # in parallel the moment the collective completes
                        nc.sync.dma_start(out=kvall[0:GAP1, :],
                                          in_=kvgather[0, 0:GAP1, :])
                        nc.scalar.dma_start(out=kvall[GAP1:P, :],
                                            in_=kvgather[1, GAP1:P, :])
                    elif (c, j) == (3, 0):
                        # wout late: after the collective's transfer window so
                        # these 3.9MB don't congest the exchange; needed only
                        # ~10us into phase 2.
                        for i in range(NJ):
                            nc.sync.dma_start(out=wout_sb[i][:, :], in_=woutT[ts(i, P), :])
                        nc.sync.dma_start(out=boutb_sb[:, :], in_=boutb)

                with tc.tile_pool(name="pstr", bufs=2, space="PSUM") as pstr:
                    for c in range(NCH):
                        for j in range(NJ):
                            if c == 3 and j >= 5:
                                continue  # deferred into phase 2
                            ps = psq.tile([P, 512], F32, tag="qps", name="qps")
                            for i in range(NJ):
                                nc.tensor.matmul(
                                    ps[:, :], wq_sb[j][:, ts(i, P)], hsT_sb[i][:, ts(c, 512)],
                                    start=(i == 0), stop=(i == NJ - 1),
                                )
                            nc.scalar.copy(qT_sb[j][:, ts(c, 512)], ps[:, :])
                            hooks(c, j)
                    # kT via PE transposes at the tail of the q stream: they
                    # wait only on the gather reads, cost ~128 cycles each,
                    # and run exactly where the PE would otherwise idle
                    # waiting out the exchange.  Copies alternate ACT/DVE.
                    for t in range(NJ):
                        tp = pstr.tile([P, P], R, tag="tps", name="tps")
                        nc.tensor.matmul(tp[:, :], kvall[:, ts(t, P)],
                                         ident16[:, :], is_transpose=True)
                        if t % 2 == 0:
                            nc.scalar.copy(kT_sb[t][:, :], tp[:, :])
                        else:
                            nc.vector.tensor_copy(kT_sb[t][:, :], tp[:, :])

            # ---- phase 2: attention + output projection, interleaved ----------
            with (
                tc.tile_pool(name="probs", bufs=3) as probs_pool,
                tc.tile_pool(name="bcp", bufs=2) as bc_pool,
                tc.tile_pool(name="finp", bufs=3) as finp,
                tc.tile_pool(name="pss", bufs=2, space="PSUM") as pss,
                tc.tile_pool(name="pso", bufs=1, space="PSUM") as pso,
                tc.tile_pool(name="psf", bufs=2, space="PSUM") as psf,
            ):
                pairs = [(c, hp) for c in range(NCH) for hp in range(NJ)]
                astate = {}

                def attn_front(idx):
                    c, hp = pairs[idx]
                    # both heads' scores in one 2-bank psum tile: one WAR
                    # wait for the pair, so the two row-group matmuls can
                    # overlap in the array.
                    ps_s = pss.tile([P, 1024], F32, tag="sps", name="sps")
                    for s in range(2):
                        rq = DH * s
                        nc.tensor.matmul(
                            ps_s[:, ts(s, 512)], kT_sb[hp][rq:rq + DH, :],
                            qT_sb[hp][rq:rq + DH, ts(c, 512)],
                            start=True, stop=True,
                        )
                    # exp in two 512-col halves: halves the scores->probs->PV
                    # chain latency (the exp is ~1.9us, the longest link)
                    probsT = probs_pool.tile([P, 1024], R, tag="probsT", name="probsT")
                    for s in range(2):
                        nc.scalar.activation(
                            probsT[:, ts(s, 512)], ps_s[:, ts(s, 512)], EXP,
                            bias=bias_col[:, :], scale=SCALE,
                        )
                    astate[idx] = [probsT[:, 0:512], probsT[:, 512:1024]]

                def attn_back(idx):
                    c, hp = pairs[idx]
                    probs = astate.pop(idx)
                    # PV + denominator of both heads in one 2-bank psum tile
                    # (cols 0:512 = PV, 512:1024 = ones-denominator): a single
                    # WAR wait for the 4-matmul group, alternating col strips
                    # so consecutive matmuls overlap in the array.
                    ps_o = pso.tile([P, 1024], F32, tag="ops", name="ops")
                    # ones-denominator matmuls FIRST so the reciprocal runs
                    # on DVE while the PV matmuls are still streaming,
                    # shortening the per-pair normalize chain.
                    for s in range(2):
                        rq = DH * s
                        nc.tensor.matmul(
                            ps_o[rq:rq + DH, 512:1024], ones_mat[:, 0:DH], probs[s][:, :],
                            start=True, stop=True,
                        )
                    for s in range(2):
                        h = 2 * hp + s
                        rq = DH * s
                        nc.tensor.matmul(
                            ps_o[rq:rq + DH, 0:512], v_sb[:, ts(h, DH)], probs[s][:, :],
                            start=True, stop=True,
                        )
                    bc_sb = bc_pool.tile([P, 512], F32, tag="bc", name="bc_sb")
                    nc.vector.reciprocal_approx_fast(bc_sb[:, :], ps_o[:, 512:1024])
                    nc.vector.tensor_mul(
                        qT_sb[hp][:, ts(c, 512)], ps_o[:, 0:512], bc_sb[:, :]
                    )

                # out-projection emitted in per-m-chunk units (10 matmuls +
                # bias-add + store each) so the work spreads evenly between
                # attention pairs instead of bunching per tile.
                fins = {}

                def out_unit(t, mi):
                    if mi == 0:
                        fins[t] = finp.tile([P, H], R, tag="fin", name="fin")
                    fin = fins[t]
                    m0, mw = MCHUNKS[mi]
                    pf = psf.tile([P, mw], F32, tag="psf", name="psf")
                    for i in range(NJ):
                        nc.tensor.matmul(
                            pf[:, :], qT_sb[i][:, ts(t, P)],
                            wout_sb[i][:, m0:m0 + mw],
                            start=(i == 0), stop=(i == NJ - 1),
                        )
                    nc.vector.tensor_add(
                        fin[:, m0:m0 + mw], pf[:, :], boutb_sb[:, m0:m0 + mw]
                    )
                    nc.sync.dma_start(out=out[ts(t, P), m0:m0 + mw],
                                      in_=fin[:, m0:m0 + mw])
                    if mi == 2:
                        del fins[t]

                def q_tail_group(j):
                    # deferred chunk-3 j-group: fills the unit-less start of
                    # phase 2 with PE work so the exp/normalize chain never
                    # outruns the stream; psf is idle here (no units yet).
                    ps = psf.tile([P, 512], F32, tag="psf", name="qtail")
                    for i in range(NJ):
                        nc.tensor.matmul(
                            ps[:, :], wq_sb[j][:, ts(i, P)], hsT_sb[i][:, 1536:2048],
                            start=(i == 0), stop=(i == NJ - 1),
                        )
                    nc.scalar.copy(qT_sb[j][:, 1536:2048], ps[:, :])

                ready_units = []
                for idx in range(len(pairs)):
                    attn_front(idx)
                    if idx < 5:
                        q_tail_group(5 + idx)
                    if idx >= 1:
                        attn_back(idx - 1)
                        pc, php = pairs[idx - 1]
                        if php == NJ - 1:
                            ready_units.extend((t, mi)
                                               for t in range(4 * pc, 4 * pc + 4)
                                               for mi in range(3))
                    # smoothed drain (12 units per 10 pairs): every pair
                    # step keeps >=1 unit of PE work so the normalize chain
                    # never outruns the stream mid-chunk
                    for _ in range(2 if idx % 5 == 0 else 1):
                        if ready_units:
                            out_unit(*ready_units.pop(0))
                attn_back(len(pairs) - 1)
                ready_units.extend((t, mi) for t in range(12, 16) for mi in range(3))
                for u in ready_units:
                    out_unit(*u)

    nc.compile()
    _NC_CACHE["nc"] = nc
    return nc


def prep_core_inputs(hidden_states, encoder_hidden_states, id_embedding,
                     Wq, Wk, Wv, Wid_k, Wid_v, Wout, bout):
    """Host-side sharding / layout prep.  Returns list of 8 in_maps."""
    f = np.float32
    h16 = np.float16
    hidden_states = np.asarray(hidden_states, f)
    encoder_hidden_states = np.asarray(encoder_hidden_states, f)
    id_embedding = np.asarray(id_embedding, f)
    Wq = np.asarray(Wq, f)
    Wout = np.asarray(Wout, f)
    Wk, Wv = np.asarray(Wk, f), np.asarray(Wv, f)
    Wid_k, Wid_v = np.asarray(Wid_k, f), np.asarray(Wid_v, f)
    boutb = np.ascontiguousarray(np.broadcast_to(np.asarray(bout, f), (P, H)))

    # packed mega-tile weight layouts: one contiguous DMA per group, with
    # [128-partition, i-major] free dims so per-i slices are plain column
    # ranges in SBUF.
    wqp = np.ascontiguousarray(
        Wq.reshape(NJ, P, NJ, P).transpose(2, 1, 0, 3).reshape(NJ, P, H)
        .astype(h16))                                                          # [j][p, i*128+m]
    wkv = np.concatenate([Wk, Wv], axis=1)                                     # [C, 2H]
    widkv = np.concatenate([Wid_k, Wid_v], axis=1)
    wkvp = np.ascontiguousarray(
        wkv.reshape(NI, P, 10, 256).transpose(2, 1, 0, 3).reshape(10, P, NI * 256)
        .astype(h16))                                                          # [n][p, i*256+m]
    widkvp = np.ascontiguousarray(
        widkv.reshape(NI, P, 10, 256).transpose(2, 1, 0, 3).reshape(10, P, NI * 256)
        .astype(h16))
    # pair-split: even core streams the encoder projection weights, odd core
    # the id projection weights; results are exchanged on-device.

    wout16 = np.ascontiguousarray(Wout.astype(h16))
    in_maps = []
    for core in range(NCORES):
        b, hf = divmod(core, 2)
        hsT = np.ascontiguousarray(hidden_states[b, hf * SC:(hf + 1) * SC, :].T.astype(h16))
        xkvT = np.zeros((C, LP), h16)                                          # [C, 128]
        xkvT[:, :TE] = encoder_hidden_states[b].T
        xkvT[:, GAP1:] = id_embedding[b % 2].T
        xkvTp = np.ascontiguousarray(
            xkvT.reshape(NI, P, LP).transpose(1, 0, 2).reshape(P, NI * LP))    # [p, i*128+l]
        in_maps.append({
            "hsT": hsT, "xkvTp": xkvTp, "wqp": wqp,
            "wkvp": wkvp if core % 2 == 0 else widkvp,
            "woutT": wout16, "boutb": boutb,
        })
    return in_maps


def kernel(hidden_states, encoder_hidden_states, id_embedding,
           Wq, Wk, Wv, Wid_k, Wid_v, Wout, bout, _trace=False):
    _ensure_axon_hooks()
    from concourse.bass_utils import run_bass_kernel_spmd

    nc = build_nc()
    in_maps = prep_core_inputs(hidden_states, encoder_hidden_states, id_embedding,
                               Wq, Wk, Wv, Wid_k, Wid_v, Wout, bout)
    kwargs = {}
    if _trace:
        import concourse.bass_utils as bu
        bu.upload_artifacts = lambda tmpdir: f"local://{tmpdir}"
        kwargs["trace"] = True
    res = run_bass_kernel_spmd(nc, in_maps, core_ids=list(range(NCORES)), **kwargs)

    outp = np.empty((B, S, H), np.float32)
    for core in range(NCORES):
        b, hf = divmod(core, 2)
        outp[b, hf * SC:(hf + 1) * SC, :] = res.results[core]["out"]
    if _trace:
        kernel.last_exec_time_ns = res.exec_time_ns
        kernel.last_results = res
    return outp


# revision 43
# speedup vs baseline: 1.1692x; 1.1692x over previous
"""Trainium2 Bass kernel for the branched cross-attention processor.

Problem (full shapes):
  hidden_states [4, 4096, 1280], encoder_hidden_states [4, 77, 2048],
  id_embedding [2, 32, 2048], Wq/Wout [1280,1280], Wk/Wv/Wid_k/Wid_v
  [2048,1280], bout [1280].  20 heads, dh=64.  Output [4, 4096, 1280].

Sharding: data-parallel over (batch, seq-half): core c handles batch c//2,
query rows (c%2)*2048 : (c%2+1)*2048.  K/V (109 keys) are computed
per-core for its batch.  All queries are independent (full cross
attention), so no collectives are needed beyond the 2-core KV exchange.

Single fused pipeline (fp16 matmul operands, fp32 PSUM accumulation):

Q projection runs c-chunk-major (4 chunks of 512 query columns) with the
j-groups inside, so the first useful matmul only needs wq[0] plus the
first 512-column slice of hsT (~1.6 MB) instead of all of hsT (5.6 MB).
hsT arrives as per-(i, c0) slices followed by per-i remainders; the wq
stream rides one j ahead of consumption.  The KV projection is
PAIR-SPLIT (even core computes the encoder projection, odd core the id
projection; same SPMD code, different weight data) and exchanged with a
2-core DRAM AllGather; the 5 kv sub-chunks interleave between j-groups
paced by their weight DMAs, finishing at the end of chunk 2 so the
exchange + kT transposes hide under chunk 3's matmuls.

Attention + output projection then stream through the same PE queue.
Per head-pair: 2 row-group scores matmuls (K=64, rows 0:64 / 64:128)
into one 2-bank psum tile so they carry a single WAR wait; exp with
gap-mask bias on ACT; 2 col-group PV matmuls + 2 col-group
ones-denominator matmuls (alternating col strips 0:64 / 64:128);
reciprocal + normalize on DVE.  attnT is written back into the qT tiles
(read-then-overwrite per chunk; saves 5MB SBUF).  As soon as a chunk's
10 head-pairs finish, its 4 query tiles of the output projection are
emitted interleaved with the next chunk's attention so the
exp/normalize work hides under out-proj matmuls and the PE never idles.

Key layout: [0:77]=encoder keys, [77:96]=zero gap (exp bias -1e30),
[96:128]=id keys.
"""

import os
import sys
import types

import numpy as np

# ---------------------------------------------------------------------------
# problem constants (hardcoded; kernel.py must be self-contained)
# ---------------------------------------------------------------------------
B = 4
S = 4096
H = 1280
C = 2048
TE = 77          # encoder tokens
TI = 32          # id tokens
HEADS = 20
DH = 64          # head dim
P = 128
L = 109          # TE + TI
LP = 128         # padded key count: [0:77]=ehs, [77:96]=gap, [96:128]=id
GAP0, GAP1 = TE, P - TI   # 77, 96
SC = 2048        # seq rows per core
NJ = H // P      # 10
NI = C // P      # 16
NCH = SC // 512  # 4 sq-chunks of 512
NT = SC // P     # 16 sq-tiles of 128
SCALE = 1.0 / 8.0
NCORES = 8
MCHUNKS = [(0, 512), (512, 512), (1024, 256)]

_NC_CACHE = {}


def _ensure_axon_hooks():
    """The image's antenv lacks axon_hooks; synthesize it so NTFF profiling
    (trace=True) works when test.py asks for it.  Harmless if unused."""
    if "antenv.axon_hooks" in sys.modules:
        return
    try:
        import antenv
        from trn_agent_boot.trn_boot import _ntff_profile_via_ctypes

        hook = _ntff_profile_via_ctypes("/opt/axon/libaxon_pjrt.so")
        m = types.ModuleType("antenv.axon_hooks")
        m.get_axon_ntff_profile_hook = lambda: hook
        m.set_axon_ntff_profile_hook = lambda h: None
        sys.modules["antenv.axon_hooks"] = m
        antenv.axon_hooks = m
    except Exception:
        pass


def build_nc():
    """Build + compile the per-core Bass program (SPMD: same NEFF, 8 cores)."""
    if "nc" in _NC_CACHE:
        return _NC_CACHE["nc"]

    import concourse.bass as bass
    import concourse.tile as tile
    from concourse import bacc, mybir
    from concourse.bass import ts

    F32 = mybir.dt.float32
    R = mybir.dt.float16      # matmul operand dtype (1 cyc/row, 10-bit mantissa)
    EXP = mybir.ActivationFunctionType.Exp

    nc = bacc.Bacc("TRN2", target_bir_lowering=False, debug=False, num_devices=NCORES)

    hsT = nc.dram_tensor("hsT", [H, SC], R, kind="ExternalInput").ap()
    xkvTp = nc.dram_tensor("xkvTp", [P, NI * LP], R, kind="ExternalInput").ap()
    wqp = nc.dram_tensor("wqp", [NJ, P, H], R, kind="ExternalInput").ap()
    # per-core kv weights: even cores get [Wk|Wv], odd cores [Wid_k|Wid_v];
    # each core computes only its projection and the pair exchanges results.
    # 10 sub-chunks of 256 output columns so the weight stream arrives (and
    # the exchange can trigger) as early as possible.
    wkvp = nc.dram_tensor("wkvp", [10, P, NI * 256], R, kind="ExternalInput").ap()
    woutT = nc.dram_tensor("woutT", [H, H], R, kind="ExternalInput").ap()
    boutb = nc.dram_tensor("boutb", [P, H], F32, kind="ExternalInput").ap()
    # fp16 output: the values come out of fp32 psum and round once on the
    # final store (~5e-4 rel), halving the 10.5MB/core output write; the
    # host gather upcasts back to fp32.
    out = nc.dram_tensor("out", [SC, H], R, kind="ExternalOutput").ap()
    kvstore = nc.dram_tensor("kvstore", [P, 5 * 512], R, kind="Internal").ap()
    kvgather = nc.dram_tensor("kvgather", [2, P, 5 * 512], R, kind="Internal").ap()
    # tiny tensors for a warm-up AllGather that absorbs the CC stream's
    # one-time setup latency long before the real exchange
    ccwarm_in = nc.dram_tensor("ccwarm_in", [P, 16], R, kind="Internal").ap()
    ccwarm_out = nc.dram_tensor("ccwarm_out", [2, P, 16], R, kind="Internal").ap()

    with tile.TileContext(nc) as tc:
        with tc.tile_pool(name="pers", bufs=1) as pers:
            # ---- persistent constants / arrays --------------------------------
            ones_mat = pers.tile([P, P], R, tag="ones_mat")
            nc.vector.memset(ones_mat[:, :], 1.0)
            bias_col = pers.tile([P, 1], F32, tag="bias_col")
            # engine ops need 32-aligned start partitions: write the gap
            # as [64:96] then restore [64:77]; later writes overwrite cleanly.
            nc.vector.memset(bias_col[:, :], 0.0)
            nc.vector.memset(bias_col[64:GAP1, :], -1e30)
            nc.vector.memset(bias_col[64:GAP0, :], 0.0)
            kT_sb = [pers.tile([P, LP], R, tag=f"kT{j}", name=f"kT{j}") for j in range(NJ)]
            # merged pair-exchanged [k~|v~] (cols 0:1280 k, 1280:2560 v)
            kvall = pers.tile([P, 5 * 512], R, tag="kvall")
            v_sb = kvall[:, 1280:2560]
            # fp16 identity for PE-transpose of the kT tiles (the PE builds
            # kT right when it would otherwise stall on the exchange, ~128
            # cycles per tile vs ~1.2us per DMA-transpose issue)
            ident16 = pers.tile([P, P], R, tag="ident16")
            from concourse import masks as _masks
            _masks.make_identity(nc, ident16[:, :])
            # qT doubles as attnT in phase 2 (normalize overwrites each
            # chunk after its scores matmul has consumed it).
            qT_sb = [pers.tile([P, SC], R, tag=f"qT{j}", name=f"qT{j}") for j in range(NJ)]
            wout_sb = [pers.tile([P, H], R, tag=f"wout{i}", name=f"wout{i}") for i in range(NJ)]
            boutb_sb = pers.tile([P, H], F32, tag="boutb")

            # ---- phase 1: q projection chunk-major + kv proj, DMA-paced -------
            with (
                tc.tile_pool(name="phq", bufs=1) as phq,
                tc.tile_pool(name="wkvs", bufs=6) as wkvs,
                tc.tile_pool(name="psq", bufs=4, space="PSUM") as psq,
                tc.tile_pool(name="pskv", bufs=2, space="PSUM") as pskv,
            ):
                hsT_sb = [phq.tile([P, SC], R, tag=f"hsT{i}", name=f"hsT{i}") for i in range(NJ)]
                wq_sb = [phq.tile([P, H], R, tag=f"wq{j}", name=f"wq{j}") for j in range(NJ)]
                xkv_all = phq.tile([P, NI * LP], R, tag="xkv_all")
                kvmine = phq.tile([P, 5 * 512], R, tag="kvmine")
                warm_mov = phq.tile([P, 512], R, tag="warm_mov")
                nc.vector.memset(warm_mov[:, :], 0.0)

                # PE warmup: ~8 matmuls on constants lift the PE p-state /
                # HAM clock gate while the first input DMAs land.  The psum
                # tile is never read; psq reuse orders it before q group 0.
                wps = psq.tile([P, 512], F32, tag="qps", name="warm_ps")
                for w in range(8):
                    nc.tensor.matmul(wps[:, :], ones_mat[:, :], warm_mov[:, :],
                                     start=(w == 0), stop=(w == 7))

                # DMA issue order is (approximate) arrival order.  Critical
                # path first: wq0/wq1 + the c0 slices of hsT so the first
                # j-groups start ~9us in; then the remaining wq stream, the
                # hsT remainders, and the kv weights behind them.
                nc.sync.dma_start(out=wq_sb[0][:, :], in_=wqp[0])
                nc.sync.dma_start(out=wq_sb[1][:, :], in_=wqp[1])
                for i in range(NJ):
                    nc.sync.dma_start(out=hsT_sb[i][:, 0:512], in_=hsT[ts(i, P), 0:512])
                for j in range(2, NJ):
                    nc.sync.dma_start(out=wq_sb[j][:, :], in_=wqp[j])
                for i in range(NJ):
                    nc.sync.dma_start(out=hsT_sb[i][:, 512:1024], in_=hsT[ts(i, P), 512:1024])
                nc.sync.dma_start(out=xkv_all[:, :], in_=xkvTp)

                kv_tiles = {}

                def issue_kv(n):
                    t = wkvs.tile([P, NI * 256], R, tag="wkv", name=f"wkv{n}")
                    nc.sync.dma_start(out=t[:, :], in_=wkvp[n])
                    kv_tiles[n] = t

                issue_kv(0)
                issue_kv(1)
                issue_kv(2)
                issue_kv(3)
                issue_kv(4)
                issue_kv(5)
                for i in range(NJ):
                    nc.sync.dma_start(out=hsT_sb[i][:, 1024:SC], in_=hsT[ts(i, P), 1024:SC])

                def kv_sub(n):
                    kvw = kv_tiles[n]
                    ps = pskv.tile([P, 256], F32, tag="kvps", name="kvps")
                    for i in range(NI):
                        nc.tensor.matmul(
                            ps[:, :], xkv_all[:, ts(i, LP)], kvw[:, ts(i, 256)],
                            start=(i == 0), stop=(i == NI - 1),
                        )
                    # evacuate on DVE (idle in phase 1) so the store isn't
                    # queued behind ACT's qT copies
                    nc.vector.tensor_copy(kvmine[:, ts(n, 256)], ps[:, :])
                    # stream my projection to HBM incrementally so the
                    # exchange can start right after the last sub-chunk
                    nc.sync.dma_start(out=kvstore[:, ts(n, 256)],
                                      in_=kvmine[:, ts(n, 256)])
                    if n + 6 < 10:
                        issue_kv(n + 6)

                # per-(c, j) hooks: all 10 kv sub-chunks run inside chunk 1
                # (their 256-col weight mega-DMAs arrive ~27-57us), so the
                # exchange triggers as early as the PE pace allows (~85us)
                # and the collective (high run-to-run latency variance)
                # completes before the q stream runs dry at ~122us.
                def hooks(c, j):
                    if (c, j) == (0, 1):
                        # warm-up AllGather on 4KB: absorbs the CC stream's
                        # ~11us one-time trigger/setup latency early, off the
                        # critical path
                        nc.gpsimd.collective_compute(
                            "AllGather", mybir.AluOpType.bypass,
                            replica_groups=[[0, 1], [2, 3], [4, 5], [6, 7]],
                            ins=[ccwarm_in], outs=[ccwarm_out],
                        )
                    elif c == 1:
                        kv_sub(j)
                        if j == 9:
                            # 2-core AllGather: slot 0 = even core ([Wk|Wv]
                            # proj, valid rows 0:96), slot 1 = odd ([Wid_*],
                            # rows 96:128)
                            nc.gpsimd.collective_compute(
                                "AllGather", mybir.AluOpType.bypass,
                                replica_groups=[[0, 1], [2, 3], [4, 5], [6, 7]],
                                ins=[kvstore], outs=[kvgather],
                            )
                    elif (c, j) == (2, 1):
                        # split across DGE queue sets so the two reads run
                        # in parallel the moment the collective completes
                        nc.sync.dma_start(out=kvall[0:GAP1, :],
                                          in_=kvgather[0, 0:GAP1, :])
                        nc.scalar.dma_start(out=kvall[GAP1:P, :],
                                            in_=kvgather[1, GAP1:P, :])
                    elif (c, j) == (3, 0):
                        # wout late: after the collective's transfer window so
                        # these 3.9MB don't congest the exchange; needed only
                        # ~10us into phase 2.
                        for i in range(NJ):
                            nc.sync.dma_start(out=wout_sb[i][:, :], in_=woutT[ts(i, P), :])
                        nc.sync.dma_start(out=boutb_sb[:, :], in_=boutb)

                with tc.tile_pool(name="pstr", bufs=2, space="PSUM") as pstr:
                    for c in range(NCH):
                        for j in range(NJ):
                            ps = psq.tile([P, 512], F32, tag="qps", name="qps")
                            for i in range(NJ):
                                nc.tensor.matmul(
                                    ps[:, :], wq_sb[j][:, ts(i, P)], hsT_sb[i][:, ts(c, 512)],
                                    start=(i == 0), stop=(i == NJ - 1),
                                )
                            nc.scalar.copy(qT_sb[j][:, ts(c, 512)], ps[:, :])
                            hooks(c, j)
                    # kT via PE transposes at the tail of the q stream: they
                    # wait only on the gather reads, cost ~128 cycles each,
                    # and run exactly where the PE would otherwise idle
                    # waiting out the exchange.  Copies alternate ACT/DVE.
                    for t in range(NJ):
                        tp = pstr.tile([P, P], R, tag="tps", name="tps")
                        nc.tensor.matmul(tp[:, :], kvall[:, ts(t, P)],
                                         ident16[:, :], is_transpose=True)
                        if t % 2 == 0:
                            nc.scalar.copy(kT_sb[t][:, :], tp[:, :])
                        else:
                            nc.vector.tensor_copy(kT_sb[t][:, :], tp[:, :])

            # ---- phase 2: attention + output projection, interleaved ----------
            with (
                tc.tile_pool(name="probs", bufs=3) as probs_pool,
                tc.tile_pool(name="bcp", bufs=2) as bc_pool,
                tc.tile_pool(name="finp", bufs=3) as finp,
                tc.tile_pool(name="pss", bufs=2, space="PSUM") as pss,
                tc.tile_pool(name="pso", bufs=1, space="PSUM") as pso,
                tc.tile_pool(name="psf", bufs=2, space="PSUM") as psf,
            ):
                pairs = [(c, hp) for c in range(NCH) for hp in range(NJ)]
                astate = {}

                def attn_front(idx):
                    c, hp = pairs[idx]
                    # both heads' scores in one 2-bank psum tile: one WAR
                    # wait for the pair, so the two row-group matmuls can
                    # overlap in the array.
                    ps_s = pss.tile([P, 1024], F32, tag="sps", name="sps")
                    for s in range(2):
                        rq = DH * s
                        nc.tensor.matmul(
                            ps_s[:, ts(s, 512)], kT_sb[hp][rq:rq + DH, :],
                            qT_sb[hp][rq:rq + DH, ts(c, 512)],
                            start=True, stop=True,
                        )
                    # exp in two 512-col halves: halves the scores->probs->PV
                    # chain latency (the exp is ~1.9us, the longest link)
                    probsT = probs_pool.tile([P, 1024], R, tag="probsT", name="probsT")
                    for s in range(2):
                        nc.scalar.activation(
                            probsT[:, ts(s, 512)], ps_s[:, ts(s, 512)], EXP,
                            bias=bias_col[:, :], scale=SCALE,
                        )
                    astate[idx] = [probsT[:, 0:512], probsT[:, 512:1024]]

                def attn_back(idx):
                    c, hp = pairs[idx]
                    probs = astate.pop(idx)
                    # PV + denominator of both heads in one 2-bank psum tile
                    # (cols 0:512 = PV, 512:1024 = ones-denominator): a single
                    # WAR wait for the 4-matmul group, alternating col strips
                    # so consecutive matmuls overlap in the array.
                    ps_o = pso.tile([P, 1024], F32, tag="ops", name="ops")
                    # ones-denominator matmuls FIRST so the reciprocal runs
                    # on DVE while the PV matmuls are still streaming,
                    # shortening the per-pair normalize chain.
                    for s in range(2):
                        rq = DH * s
                        nc.tensor.matmul(
                            ps_o[rq:rq + DH, 512:1024], ones_mat[:, 0:DH], probs[s][:, :],
                            start=True, stop=True,
                        )
                    for s in range(2):
                        h = 2 * hp + s
                        rq = DH * s
                        nc.tensor.matmul(
                            ps_o[rq:rq + DH, 0:512], v_sb[:, ts(h, DH)], probs[s][:, :],
                            start=True, stop=True,
                        )
                    bc_sb = bc_pool.tile([P, 512], F32, tag="bc", name="bc_sb")
                    nc.vector.reciprocal_approx_fast(bc_sb[:, :], ps_o[:, 512:1024])
                    nc.vector.tensor_mul(
                        qT_sb[hp][:, ts(c, 512)], ps_o[:, 0:512], bc_sb[:, :]
                    )

                # out-projection emitted in per-m-chunk units (10 matmuls +
                # bias-add + store each) so the work spreads evenly between
                # attention pairs instead of bunching per tile.
                fins = {}

                def out_unit(t, mi):
                    if mi == 0:
                        fins[t] = finp.tile([P, H], R, tag="fin", name="fin")
                    fin = fins[t]
                    m0, mw = MCHUNKS[mi]
                    pf = psf.tile([P, mw], F32, tag="psf", name="psf")
                    for i in range(NJ):
                        nc.tensor.matmul(
                            pf[:, :], qT_sb[i][:, ts(t, P)],
                            wout_sb[i][:, m0:m0 + mw],
                            start=(i == 0), stop=(i == NJ - 1),
                        )
                    nc.vector.tensor_add(
                        fin[:, m0:m0 + mw], pf[:, :], boutb_sb[:, m0:m0 + mw]
                    )
                    nc.sync.dma_start(out=out[ts(t, P), m0:m0 + mw],
                                      in_=fin[:, m0:m0 + mw])
                    if mi == 2:
                        del fins[t]

                ready_units = []
                for idx in range(len(pairs)):
                    attn_front(idx)
                    if idx >= 1:
                        attn_back(idx - 1)
                        pc, php = pairs[idx - 1]
                        if php == NJ - 1:
                            ready_units.extend((t, mi)
                                               for t in range(4 * pc, 4 * pc + 4)
                                               for mi in range(3))
                    # smoothed drain (12 units per 10 pairs): every pair
                    # step keeps >=1 unit of PE work so the normalize chain
                    # never outruns the stream mid-chunk
                    for _ in range(2 if idx % 5 == 0 else 1):
                        if ready_units:
                            out_unit(*ready_units.pop(0))
                attn_back(len(pairs) - 1)
                ready_units.extend((t, mi) for t in range(12, 16) for mi in range(3))
                for u in ready_units:
                    out_unit(*u)

    nc.compile()
    _NC_CACHE["nc"] = nc
    return nc


def prep_core_inputs(hidden_states, encoder_hidden_states, id_embedding,
                     Wq, Wk, Wv, Wid_k, Wid_v, Wout, bout):
    """Host-side sharding / layout prep.  Returns list of 8 in_maps."""
    f = np.float32
    h16 = np.float16
    hidden_states = np.asarray(hidden_states, f)
    encoder_hidden_states = np.asarray(encoder_hidden_states, f)
    id_embedding = np.asarray(id_embedding, f)
    Wq = np.asarray(Wq, f)
    Wout = np.asarray(Wout, f)
    Wk, Wv = np.asarray(Wk, f), np.asarray(Wv, f)
    Wid_k, Wid_v = np.asarray(Wid_k, f), np.asarray(Wid_v, f)
    boutb = np.ascontiguousarray(np.broadcast_to(np.asarray(bout, f), (P, H)))

    # packed mega-tile weight layouts: one contiguous DMA per group, with
    # [128-partition, i-major] free dims so per-i slices are plain column
    # ranges in SBUF.
    wqp = np.ascontiguousarray(
        Wq.reshape(NJ, P, NJ, P).transpose(2, 1, 0, 3).reshape(NJ, P, H)
        .astype(h16))                                                          # [j][p, i*128+m]
    wkv = np.concatenate([Wk, Wv], axis=1)                                     # [C, 2H]
    widkv = np.concatenate([Wid_k, Wid_v], axis=1)
    wkvp = np.ascontiguousarray(
        wkv.reshape(NI, P, 10, 256).transpose(2, 1, 0, 3).reshape(10, P, NI * 256)
        .astype(h16))                                                          # [n][p, i*256+m]
    widkvp = np.ascontiguousarray(
        widkv.reshape(NI, P, 10, 256).transpose(2, 1, 0, 3).reshape(10, P, NI * 256)
        .astype(h16))
    # pair-split: even core streams the encoder projection weights, odd core
    # the id projection weights; results are exchanged on-device.

    wout16 = np.ascontiguousarray(Wout.astype(h16))
    in_maps = []
    for core in range(NCORES):
        b, hf = divmod(core, 2)
        hsT = np.ascontiguousarray(hidden_states[b, hf * SC:(hf + 1) * SC, :].T.astype(h16))
        xkvT = np.zeros((C, LP), h16)                                          # [C, 128]
        xkvT[:, :TE] = encoder_hidden_states[b].T
        xkvT[:, GAP1:] = id_embedding[b % 2].T
        xkvTp = np.ascontiguousarray(
            xkvT.reshape(NI, P, LP).transpose(1, 0, 2).reshape(P, NI * LP))    # [p, i*128+l]
        in_maps.append({
            "hsT": hsT, "xkvTp": xkvTp, "wqp": wqp,
            "wkvp": wkvp if core % 2 == 0 else widkvp,
            "woutT": wout16, "boutb": boutb,
        })
    return in_maps


def kernel(hidden_states, encoder_hidden_states, id_embedding,
           Wq, Wk, Wv, Wid_k, Wid_v, Wout, bout, _trace=False):
    _ensure_axon_hooks()
    from concourse.bass_utils import run_bass_kernel_spmd

    nc = build_nc()
    in_maps = prep_core_inputs(hidden_states, encoder_hidden_states, id_embedding,
                               Wq, Wk, Wv, Wid_k, Wid_v, Wout, bout)
    kwargs = {}
    if _trace:
        import concourse.bass_utils as bu
        bu.upload_artifacts = lambda tmpdir: f"local://{tmpdir}"
        kwargs["trace"] = True
    res = run_bass_kernel_spmd(nc, in_maps, core_ids=list(range(NCORES)), **kwargs)

    outp = np.empty((B, S, H), np.float32)
    for core in range(NCORES):
        b, hf = divmod(core, 2)
        outp[b, hf * SC:(hf + 1) * SC, :] = res.results[core]["out"]
    if _trace:
        kernel.last_exec_time_ns = res.exec_time_ns
        kernel.last_results = res
    return outp
